# revision 8
# baseline (speedup 1.0000x reference)
"""Causal self-attention on 8 Trainium2 NeuronCores.

Sharding (matches the batch+head-parallel hint): core c handles batch
b = c // 4 and the 4 heads [hg*4, hg*4+4) where hg = c % 4.  Each core
computes its q/k/v projections from column-sliced c_attn weights, full
causal attention for its heads, and a partial c_proj output from the
matching row slice of w_proj; the host sums the 4 partials per batch.

All matmuls run in float32r (TF32-like rounding, fp32 accumulate).
"""

import sys

if "/opt/trn_rl_repo" not in sys.path:
    sys.path.insert(0, "/opt/trn_rl_repo")

import numpy as np

import concourse.mybir as mybir
from concourse import bacc
from concourse.bass_utils import run_bass_kernel_spmd
from concourse.tile import TileContext

B, T, C = 2, 2048, 1024
H, D = 16, 64
HL = 4  # heads per core
N_CORES = 8
KT = C // 128  # contraction tiles over the embedding dim
SCALE = 1.0 / 8.0  # 1/sqrt(D)

_CACHE = {}


def _build():
    f32 = mybir.dt.float32
    f32r = mybir.dt.float32r
    nc = bacc.Bacc("TRN2", target_bir_lowering=False, debug=False, num_devices=N_CORES)

    x_in = nc.dram_tensor("x_in", [128, KT, T], f32r, kind="ExternalInput")
    wqk = nc.dram_tensor("wqk", [128, KT, 2 * HL * D], f32r, kind="ExternalInput")
    wv = nc.dram_tensor("wv", [128, KT, HL * D], f32r, kind="ExternalInput")
    wp = nc.dram_tensor("wp", [D, HL, C], f32r, kind="ExternalInput")
    out = nc.dram_tensor("out", [T, C], f32, kind="ExternalOutput")

    with TileContext(nc) as tc:
        with tc.tile_pool(name="persist", bufs=1) as persist:
            # q/k feature-major: slot 0/1 = q heads {0,1}/{2,3}, slot 2/3 = k
            qk_sb = persist.tile([128, 4, T], f32r)
            # v token-major, 65th column holds ones for the softmax denominator
            v_sb = persist.tile([128, 16, HL, D + 1], f32r)
            yT = persist.tile([D, HL, T], f32r)
            wp_sb = persist.tile([D, HL, C], f32r)
            nc.sync.dma_start(wp_sb, wp[:, :, :])
            ones32 = persist.tile([128, HL, 1], f32)
            nc.vector.memset(ones32, 1.0)
            for tt in range(16):
                nc.vector.tensor_copy(v_sb[:, tt, :, D : D + 1], ones32)
            # ones row at partition D, used to broadcast the softmax
            # reciprocal across partitions via a K=1 matmul
            onesbc32 = persist.tile([D + 1, D], f32)
            nc.vector.memset(onesbc32[D : D + 1, :], 1.0)
            onesbc = persist.tile([D + 1, D], f32r)
            nc.vector.tensor_copy(onesbc[D : D + 1, :], onesbc32[D : D + 1, :])

            # ---- QKV projections ----
            with (
                tc.tile_pool(name="qkvp", bufs=1) as qkvp,
                tc.tile_pool(name="ps_qk", bufs=3, space="PSUM") as ps_qk,
                tc.tile_pool(name="ps_v", bufs=3, space="PSUM") as ps_v,
            ):
                x_sb = qkvp.tile([128, KT, T], f32r)
                wqk_sb = qkvp.tile([128, KT, 2 * HL * D], f32r)
                wv_sb = qkvp.tile([128, KT, HL * D], f32r)
                nc.sync.dma_start(wqk_sb, wqk[:, :, :])
                nc.sync.dma_start(wv_sb, wv[:, :, :])
                for kt in range(KT):
                    nc.sync.dma_start(x_sb[:, kt, :], x_in[:, kt, :])

                for jt in range(4):
                    for tb in range(4):
                        qk_ps = ps_qk.tile([128, 512], f32)
                        for kt in range(KT):
                            nc.tensor.matmul(
                                qk_ps,
                                wqk_sb[:, kt, jt * 128 : (jt + 1) * 128],
                                x_sb[:, kt, tb * 512 : (tb + 1) * 512],
                                start=(kt == 0),
                                stop=(kt == KT - 1),
                            )
                        nc.vector.tensor_copy(
                            qk_sb[:, jt, tb * 512 : (tb + 1) * 512], qk_ps
                        )
                for tt in range(16):
                    v_ps = ps_v.tile([128, HL * D], f32)
                    for kt in range(KT):
                        nc.tensor.matmul(
                            v_ps,
                            x_sb[:, kt, tt * 128 : (tt + 1) * 128],
                            wv_sb[:, kt, :],
                            start=(kt == 0),
                            stop=(kt == KT - 1),
                        )
                    nc.vector.tensor_copy(
                        v_sb[:, tt, :, 0:D],
                        v_ps.rearrange("p (h d) -> p h d", h=HL),
                    )

            # ---- causal attention ----
            with (
                tc.tile_pool(name="attp", bufs=4) as attp,
                tc.tile_pool(name="attsmall", bufs=4) as attsmall,
                tc.tile_pool(name="ps_st", bufs=4, space="PSUM") as ps_st,
                tc.tile_pool(name="ps_y", bufs=2, space="PSUM") as ps_y,
                tc.tile_pool(name="ps_rb", bufs=2, space="PSUM") as ps_rb,
            ):
                for h in range(HL):
                    qslot = h // 2
                    kslot = 2 + h // 2
                    base = (h % 2) * D
                    for jq in range(4):
                        y_ps = ps_y.tile([D + 1, 512], f32)
                        njt = 4 * (jq + 1)
                        for j in range(njt):
                            w = max(0, (j - 4 * jq) * 128)
                            # S^T tile [tk=128, tq=512-w]
                            st_ps = ps_st.tile([128, 512], f32)
                            nc.tensor.matmul(
                                st_ps[:, w:],
                                qk_sb[base : base + D, kslot, j * 128 : (j + 1) * 128],
                                qk_sb[
                                    base : base + D,
                                    qslot,
                                    jq * 512 + w : (jq + 1) * 512,
                                ],
                                start=True,
                                stop=True,
                            )
                            est = attp.tile([128, 512], f32r, tag="est")
                            nc.scalar.activation(
                                est[:, w:],
                                st_ps[:, w:],
                                mybir.ActivationFunctionType.Exp,
                                scale=SCALE,
                            )
                            if j >= 4 * jq:
                                # zero the strictly-upper triangle of the
                                # 128x128 block straddling the diagonal
                                nc.gpsimd.affine_select(
                                    out=est[:, w : w + 128],
                                    in_=est[:, w : w + 128],
                                    pattern=[[1, 128]],
                                    channel_multiplier=-1,
                                    base=0,
                                    compare_op=mybir.AluOpType.is_ge,
                                    fill=0.0,
                                )
                            nc.tensor.matmul(
                                y_ps[:, w:],
                                v_sb[:, j, h, :],
                                est[:, w:],
                                start=(j == 0),
                                stop=(j == njt - 1),
                            )
                        # normalize: row D of y_ps is the softmax denominator
                        rr = attsmall.tile([D + 1, 512], f32r, tag="rr")
                        with nc.allow_low_precision(reason="f32r is fp32-width"):
                            nc.vector.reciprocal(rr[D : D + 1, :], y_ps[D : D + 1, :])
                        rb_ps = ps_rb.tile([D, 512], f32)
                        nc.tensor.matmul(
                            rb_ps,
                            onesbc[D : D + 1, :],
                            rr[D : D + 1, :],
                            start=True,
                            stop=True,
                        )
                        rb_sb = attsmall.tile([D, 512], f32, tag="rb")
                        nc.vector.tensor_copy(rb_sb, rb_ps)
                        nc.vector.tensor_mul(
                            yT[:, h, jq * 512 : (jq + 1) * 512], y_ps[0:D, :], rb_sb
                        )

            # ---- output projection (partial over this core's heads) ----
            with (
                tc.tile_pool(name="projp", bufs=3) as projp,
                tc.tile_pool(name="ps_o", bufs=3, space="PSUM") as ps_o,
            ):
                for tt in range(16):
                    o_sb = projp.tile([128, C], f32)
                    for cb in range(2):
                        o_ps = ps_o.tile([128, 512], f32)
                        for h in range(HL):
                            nc.tensor.matmul(
                                o_ps,
                                yT[:, h, tt * 128 : (tt + 1) * 128],
                                wp_sb[:, h, cb * 512 : (cb + 1) * 512],
                                start=(h == 0),
                                stop=(h == HL - 1),
                            )
                        nc.vector.tensor_copy(o_sb[:, cb * 512 : (cb + 1) * 512], o_ps)
                    nc.sync.dma_start(out[tt * 128 : (tt + 1) * 128, :], o_sb)

    nc.compile()
    return nc


def _get_nc():
    if "nc" not in _CACHE:
        _CACHE["nc"] = _build()
    return _CACHE["nc"]


def make_in_maps(x, w_attn, w_proj):
    x = np.asarray(x, np.float32)
    w_attn = np.asarray(w_attn, np.float32)
    w_proj = np.asarray(w_proj, np.float32)
    in_maps = []
    for c in range(N_CORES):
        b, hg = c // 4, c % 4
        hs = hg * HL * D  # 256 * hg
        xt = np.ascontiguousarray(x[b].T)  # [C, T]
        x_t = xt.reshape(KT, 128, T).transpose(1, 0, 2)
        wq = w_attn[hs : hs + HL * D, :]
        wk = w_attn[C + hs : C + hs + HL * D, :]
        wqkt = np.concatenate([wq, wk], 0).T  # [C, 512]
        wqk_t = wqkt.reshape(KT, 128, 2 * HL * D).transpose(1, 0, 2)
        wvt = w_attn[2 * C + hs : 2 * C + hs + HL * D, :].T  # [C, 256]
        wv_t = wvt.reshape(KT, 128, HL * D).transpose(1, 0, 2)
        wp_t = (
            w_proj[:, hs : hs + HL * D].T.reshape(HL, D, C).transpose(1, 0, 2)
        )  # [D, HL, C]
        in_maps.append(
            {
                "x_in": np.ascontiguousarray(x_t, np.float32),
                "wqk": np.ascontiguousarray(wqk_t, np.float32),
                "wv": np.ascontiguousarray(wv_t, np.float32),
                "wp": np.ascontiguousarray(wp_t, np.float32),
            }
        )
    return in_maps


def run(in_maps, **kwargs):
    nc = _get_nc()
    return run_bass_kernel_spmd(nc, in_maps, core_ids=list(range(N_CORES)), **kwargs)


def combine(results):
    out = np.zeros((B, T, C), np.float64)
    for c in range(N_CORES):
        out[c // 4] += results[c]["out"].astype(np.float64)
    return out.astype(np.float32)


def kernel(x, w_attn, w_proj):
    res = run(make_in_maps(x, w_attn, w_proj))
    return combine(res.results)


# revision 14
# speedup vs baseline: 1.1380x; 1.1380x over previous
"""Causal self-attention on 8 Trainium2 NeuronCores.

Sharding (matches the batch+head-parallel hint): core c handles batch
b = c // 4 and the 4 heads [hg*4, hg*4+4) where hg = c % 4.  Each core
computes its q/k/v projections from column-sliced c_attn weights, full
causal attention for its heads, and a partial c_proj output from the
matching row slice of w_proj; the host sums the 4 partials per batch.

All matmuls run in float32r (TF32-like rounding, fp32 accumulate).
"""

import sys

if "/opt/trn_rl_repo" not in sys.path:
    sys.path.insert(0, "/opt/trn_rl_repo")

import numpy as np

import concourse.mybir as mybir
from concourse import bacc
from concourse.bass_utils import run_bass_kernel_spmd
from concourse.tile import TileContext

B, T, C = 2, 2048, 1024
H, D = 16, 64
HL = 4  # heads per core
N_CORES = 8
KT = C // 128  # contraction tiles over the embedding dim
SCALE = 1.0 / 8.0  # 1/sqrt(D)

_CACHE = {}


def _build():
    f32 = mybir.dt.float32
    f32r = mybir.dt.float32r
    nc = bacc.Bacc("TRN2", target_bir_lowering=False, debug=False, num_devices=N_CORES)

    x_in = nc.dram_tensor("x_in", [128, KT, T], f32r, kind="ExternalInput")
    wqk = nc.dram_tensor("wqk", [128, KT, 2 * HL * D], f32r, kind="ExternalInput")
    wv = nc.dram_tensor("wv", [128, KT, HL * D], f32r, kind="ExternalInput")
    wp = nc.dram_tensor("wp", [128, HL // 2, C], f32r, kind="ExternalInput")
    out = nc.dram_tensor("out", [T, C], f32, kind="ExternalOutput")

    with TileContext(nc) as tc:
        with tc.tile_pool(name="persist", bufs=1) as persist:
            # q/k feature-major: slot 0/1 = q heads {0,1}/{2,3}, slot 2/3 = k
            qk_sb = persist.tile([128, 4, T], f32r)
            # v token-major, 65th column holds ones for the softmax denominator
            v_sb = persist.tile([128, 16, HL, D + 1], f32r)
            # head-pair stacked y: partitions 0-63 = even head, 64-127 = odd
            yT2 = persist.tile([128, HL // 2, T], f32r)
            wp_sb = persist.tile([128, HL // 2, C], f32r)
            nc.sync.dma_start(wp_sb, wp[:, :, :])
            ones32 = persist.tile([128, HL, 1], f32)
            nc.vector.memset(ones32, 1.0)
            for tt in range(16):
                nc.vector.tensor_copy(v_sb[:, tt, :, D : D + 1], ones32)
            # ones row at partition D, used to broadcast the softmax
            # denominator across partitions via a K=1 matmul
            onesbc32 = persist.tile([D + 1, D], f32)
            nc.vector.memset(onesbc32[D : D + 1, :], 1.0)
            onesbc = persist.tile([D + 1, D], f32r)
            nc.vector.tensor_copy(onesbc[D : D + 1, :], onesbc32[D : D + 1, :])
            # lower-triangular 0/1 mask for the diagonal 128x128 blocks
            tri32 = persist.tile([128, 128], f32)
            nc.vector.memset(tri32, 1.0)
            nc.gpsimd.affine_select(
                out=tri32,
                in_=tri32,
                pattern=[[1, 128]],
                channel_multiplier=-1,
                base=0,
                compare_op=mybir.AluOpType.is_ge,
                fill=0.0,
            )
            tri = persist.tile([128, 128], f32r)
            nc.vector.tensor_copy(tri, tri32)

            # ---- QKV projections ----
            with (
                tc.tile_pool(name="qkvp", bufs=1) as qkvp,
                tc.tile_pool(name="ps_qk", bufs=3, space="PSUM") as ps_qk,
                tc.tile_pool(name="ps_v", bufs=3, space="PSUM") as ps_v,
            ):
                x_sb = qkvp.tile([128, KT, T], f32r)
                wqk_sb = qkvp.tile([128, KT, 2 * HL * D], f32r)
                wv_sb = qkvp.tile([128, KT, HL * D], f32r)
                nc.sync.dma_start(wqk_sb, wqk[:, :, :])
                nc.sync.dma_start(wv_sb, wv[:, :, :])
                for kt in range(KT):
                    nc.sync.dma_start(x_sb[:, kt, :], x_in[:, kt, :])

                for jt in range(4):
                    for tb in range(4):
                        qk_ps = ps_qk.tile([128, 512], f32)
                        for kt in range(KT):
                            nc.tensor.matmul(
                                qk_ps,
                                wqk_sb[:, kt, jt * 128 : (jt + 1) * 128],
                                x_sb[:, kt, tb * 512 : (tb + 1) * 512],
                                start=(kt == 0),
                                stop=(kt == KT - 1),
                            )
                        nc.vector.tensor_copy(
                            qk_sb[:, jt, tb * 512 : (tb + 1) * 512], qk_ps
                        )
                for tt in range(16):
                    v_ps = ps_v.tile([128, HL * D], f32)
                    for kt in range(KT):
                        nc.tensor.matmul(
                            v_ps,
                            x_sb[:, kt, tt * 128 : (tt + 1) * 128],
                            wv_sb[:, kt, :],
                            start=(kt == 0),
                            stop=(kt == KT - 1),
                        )
                    nc.vector.tensor_copy(
                        v_sb[:, tt, :, 0:D],
                        v_ps.rearrange("p (h d) -> p h d", h=HL),
                    )

            # ---- causal attention ----
            with (
                tc.tile_pool(name="attp", bufs=4) as attp,
                tc.tile_pool(name="attsmall", bufs=4) as attsmall,
                tc.tile_pool(name="ps_st", bufs=4, space="PSUM") as ps_st,
                tc.tile_pool(name="ps_y", bufs=2, space="PSUM") as ps_y,
                tc.tile_pool(name="ps_rb", bufs=2, space="PSUM") as ps_rb,
            ):
                for h in range(HL):
                    qslot = h // 2
                    kslot = 2 + h // 2
                    base = (h % 2) * D
                    for jq in range(4):
                        y_ps = ps_y.tile([D + 1, 512], f32)
                        njt = 4 * (jq + 1)

                        def s_stage(j):
                            w = max(0, (j - 4 * jq) * 128)
                            # S^T tile [tk=128, tq=512-w]
                            st_ps = ps_st.tile([128, 512], f32, name="st_ps")
                            nc.tensor.matmul(
                                st_ps[:, w:],
                                qk_sb[base : base + D, kslot, j * 128 : (j + 1) * 128],
                                qk_sb[
                                    base : base + D,
                                    qslot,
                                    jq * 512 + w : (jq + 1) * 512,
                                ],
                                start=True,
                                stop=True,
                            )
                            est = attp.tile([128, 512], f32r, tag="est", name="est")
                            nc.scalar.activation(
                                est[:, w:],
                                st_ps[:, w:],
                                mybir.ActivationFunctionType.Exp,
                                scale=SCALE,
                            )
                            if j >= 4 * jq:
                                # zero the strictly-upper triangle of the
                                # 128x128 block straddling the diagonal
                                nc.vector.tensor_mul(
                                    est[:, w : w + 128], est[:, w : w + 128], tri
                                )
                            return est, w

                        def pv_stage(j, est, w):
                            nc.tensor.matmul(
                                y_ps[:, w:],
                                v_sb[:, j, h, :],
                                est[:, w:],
                                start=(j == 0),
                                stop=(j == njt - 1),
                            )

                        # software pipeline: S/exp for j+1 issued before PV(j)
                        prev = None
                        for j in range(njt):
                            cur = s_stage(j)
                            if prev is not None:
                                pv_stage(j - 1, *prev)
                            prev = cur
                        pv_stage(njt - 1, *prev)

                        # normalize: row D of y_ps is the softmax denominator
                        r_sb = attsmall.tile([D + 1, 512], f32r, tag="rr")
                        nc.scalar.copy(r_sb[D : D + 1, :], y_ps[D : D + 1, :])
                        rb_ps = ps_rb.tile([D, 512], f32)
                        nc.tensor.matmul(
                            rb_ps,
                            onesbc[D : D + 1, :],
                            r_sb[D : D + 1, :],
                            start=True,
                            stop=True,
                        )
                        rb_sb = attsmall.tile([D, 512], f32, tag="rb")
                        nc.vector.reciprocal(rb_sb, rb_ps)
                        pr = h // 2
                        if h % 2 == 0:
                            nc.vector.tensor_mul(
                                yT2[0:D, pr, jq * 512 : (jq + 1) * 512],
                                y_ps[0:D, :],
                                rb_sb,
                            )
                        else:
                            y_lo = attsmall.tile([D, 512], f32r, tag="ylo")
                            nc.vector.tensor_mul(y_lo, y_ps[0:D, :], rb_sb)
                            nc.sync.dma_start(
                                yT2[D:128, pr, jq * 512 : (jq + 1) * 512], y_lo
                            )

            # ---- output projection (partial over this core's heads) ----
            with (
                tc.tile_pool(name="projp", bufs=3) as projp,
                tc.tile_pool(name="ps_o", bufs=3, space="PSUM") as ps_o,
            ):
                npr = HL // 2
                for tt in range(16):
                    o_sb = projp.tile([128, C], f32)
                    for cb in range(2):
                        o_ps = ps_o.tile([128, 512], f32)
                        for pr in range(npr):
                            nc.tensor.matmul(
                                o_ps,
                                yT2[:, pr, tt * 128 : (tt + 1) * 128],
                                wp_sb[:, pr, cb * 512 : (cb + 1) * 512],
                                start=(pr == 0),
                                stop=(pr == npr - 1),
                            )
                        nc.vector.tensor_copy(o_sb[:, cb * 512 : (cb + 1) * 512], o_ps)
                    nc.sync.dma_start(out[tt * 128 : (tt + 1) * 128, :], o_sb)

    nc.compile()
    return nc


def _get_nc():
    if "nc" not in _CACHE:
        _CACHE["nc"] = _build()
    return _CACHE["nc"]


def make_in_maps(x, w_attn, w_proj):
    x = np.asarray(x, np.float32)
    w_attn = np.asarray(w_attn, np.float32)
    w_proj = np.asarray(w_proj, np.float32)
    in_maps = []
    for c in range(N_CORES):
        b, hg = c // 4, c % 4
        hs = hg * HL * D  # 256 * hg
        xt = np.ascontiguousarray(x[b].T)  # [C, T]
        x_t = xt.reshape(KT, 128, T).transpose(1, 0, 2)
        wq = w_attn[hs : hs + HL * D, :]
        wk = w_attn[C + hs : C + hs + HL * D, :]
        wqkt = np.concatenate([wq, wk], 0).T  # [C, 512]
        wqk_t = wqkt.reshape(KT, 128, 2 * HL * D).transpose(1, 0, 2)
        wvt = w_attn[2 * C + hs : 2 * C + hs + HL * D, :].T  # [C, 256]
        wv_t = wvt.reshape(KT, 128, HL * D).transpose(1, 0, 2)
        # head-pair stacked rows: [128, HL//2, C]; partition p of pair pr is
        # local feature pr*128 + p (head 2*pr dims then head 2*pr+1 dims)
        wp_t = (
            w_proj[:, hs : hs + HL * D].T.reshape(HL // 2, 128, C).transpose(1, 0, 2)
        )
        in_maps.append(
            {
                "x_in": np.ascontiguousarray(x_t, np.float32),
                "wqk": np.ascontiguousarray(wqk_t, np.float32),
                "wv": np.ascontiguousarray(wv_t, np.float32),
                "wp": np.ascontiguousarray(wp_t, np.float32),
            }
        )
    return in_maps


def run(in_maps, **kwargs):
    nc = _get_nc()
    return run_bass_kernel_spmd(nc, in_maps, core_ids=list(range(N_CORES)), **kwargs)


def combine(results):
    out = np.zeros((B, T, C), np.float64)
    for c in range(N_CORES):
        out[c // 4] += results[c]["out"].astype(np.float64)
    return out.astype(np.float32)


def kernel(x, w_attn, w_proj):
    res = run(make_in_maps(x, w_attn, w_proj))
    return combine(res.results)
